# revision 1
# baseline (speedup 1.0000x reference)
"""DiffiT transformer block kernel for 8 Trainium2 NeuronCores.

Data-parallel over the B=64 window axis (8 windows per core). Activations
are feature-major ([channel, token]) so every linear contracts over the
SBUF partition axis. Q/K stay feature-major with heads packed at a 96-row
stride (so each head's 72 rows sit at 32-aligned partition bases and the
per-head score matmuls can slice them legally); V is produced token-major
into per-head slots with an appended ones-column, so O^T = V_aug.T @ P^T
yields the softmax denominator as row 72. Per-token scalars (LN mean/rstd,
softmax 1/l) are broadcast across partitions with K=1 ones-matmuls on the
PE. Dense matmuls run bf16; the residual stream stays fp32; small fixup
matmuls use float32r (full-rate fp32 at free-dim >= 256).

All biases and the time-token conditioning (c @ qkvt^T + biases) enter as
rank-1 (K=1) matmul fixups folded into the PSUM accumulations.
"""

import math
from contextlib import ExitStack

import numpy as np
import ml_dtypes

import concourse.bass as bass
import concourse.mybir as mybir
import concourse.tile as tile
from concourse import bacc
from concourse import bass_utils

F32 = mybir.dt.float32
F32R = mybir.dt.float32r
BF16 = mybir.dt.bfloat16
NPBF16 = ml_dtypes.bfloat16
AF = mybir.ActivationFunctionType

P = 128
WS = 16
N = 256            # tokens per window
C = 1152           # hidden
H = 16             # heads
DH = 72            # head dim
HS = 96            # head stride in the QK packing (32-aligned, >= DH)
MLP = 4608
EPS = 1e-6
B = 64
NCORES = 8
NW = B // NCORES   # windows per core
KC = C // P        # 9  k-tiles over the hidden dim
QKM = 2 * H * HS // P   # 24 m-tiles over packed Q+K (96-stride)
KOFF = QKM // 2    # first K-side m-tile
M1T = MLP // P     # 36 fc1 row tiles
SCALE = 1.0 / math.sqrt(DH)


def _r(ap):
    """view a 4-byte fp32 AP as float32r for full-rate PE matmuls"""
    return ap.bitcast(F32R)


def _qk_pieces(h):
    """32-aligned partition pieces covering head h's 72 rows in the
    96-stride packing: [(subtile, base, length), ...]; piece legality:
    base 0 any len, base 64 len<=64, base 32/96 len<=32."""
    start, end = HS * h, HS * h + DH
    out = []
    while start < end:
        sub, base = divmod(start, P)
        ln = min(end - start, P - base)
        if base == 64:
            ln = min(ln, 64)
        elif base in (32, 96):
            ln = min(ln, 32)
        elif base != 0:
            raise AssertionError(base)
        out.append((sub, base, ln))
        start += ln
    return out


def build_program(nw=NW, sim_gelu=False):
    nc = bacc.Bacc("TRN2", target_bir_lowering=False, debug=False,
                   num_devices=NCORES)

    # register the layernorm epsilon as a const AP (activation float biases
    # other than 0.0/1.0 need one), same pattern as Bass.__init__
    eps_t = nc.alloc_sbuf_tensor("const-eps", [P, 1], F32)
    nc.gpsimd.memset(eps_t.ap(), EPS)
    nc.const_aps.aps[(F32, EPS)] = eps_t.ap()
    nc.all_engine_barrier()

    def din(name, shape, dt):
        return nc.dram_tensor(name, shape, dt, kind="ExternalInput").ap()

    xT = din("xT", [nw, P, KC, N], F32)          # x, feature-major
    xTb = din("xTb", [nw, P, KC, N], BF16)       # x, bf16 copy for LN stats
    cT = din("cT", [10, P, nw], BF16)            # c augmented with ones row
    wct = din("wct", [10, P, 4224], BF16)        # qkvt^T reordered + bias row
    wqk = din("wqk", [QKM, P, KC, P], BF16)      # qkv^T QK part, 96-stride
    wv = din("wv", [4, P, KC, 288], BF16)        # qkv^T V part, chunk-major
    expb = din("expb", [H, P, 2, N], BF16)       # exp(rel-pos bias)^T per head
    wps = din("wps", [KC, P, H, P], BF16)        # proj^T, head-slot padded
    w1c = din("w1c", [M1T, P, KC, P], BF16)      # fc1^T pre-chunked
    w2 = din("w2", [KC, P, M1T, P], BF16)        # fc2^T, pm-chunked
    f1b = din("f1b", [P, M1T], F32)              # fc1 bias, per-partition
    b2 = din("b2", [1, 2 * C], BF16)             # proj_b ++ fc2_b
    outT = nc.dram_tensor("outT", [nw, P, KC, N], F32,
                          kind="ExternalOutput").ap()

    NPAIR = nw // 2
    W2N = 2 * N        # tokens per window pair

    with tile.TileContext(nc) as tc, ExitStack() as ctx:
        keep = ctx.enter_context(tc.tile_pool(name="keep", bufs=1))
        dram = ctx.enter_context(tc.tile_pool(name="dram", bufs=1,
                                              space="DRAM"))

        ones_b = keep.tile([1, W2N], BF16, tag="ones_b")  # bf16 rhs of K=1
        ones_c = keep.tile([P, 1], BF16, tag="ones_c")    # lhsT of column sums
        nc.gpsimd.memset(ones_b[:], 1.0)
        nc.gpsimd.memset(ones_c[:], 1.0)
        bias2 = keep.tile([1, 2 * C], BF16, tag="bias2")
        nc.sync.dma_start(bias2[:], b2[:])
        f1bs = keep.tile([P, M1T], F32, tag="f1bs")
        nc.sync.dma_start(f1bs[:], f1b[:])

        tdram = dram.tile([nw, 4224], BF16)
        xpd = dram.tile([nw, P, KC, N], F32)     # x after attention branch
        xpdb = dram.tile([nw, P, KC, N], BF16)   # bf16 shadow for LN2

        # ---- phase 0: conditioning T = c_aug @ W_ct ----------------------
        with tc.tile_pool(name="ph0", bufs=2) as p0, \
             tc.tile_pool(name="ph0p", bufs=2, space="PSUM") as pp0:
            caug = p0.tile([P, 10, nw], BF16, tag="caug")
            nc.sync.dma_start(caug[:], cT.rearrange("k p w -> p k w"))
            tsb = p0.tile([8, 4224], BF16, tag="tsb")
            for i in range(9):
                n0, nl = i * 512, min(512, 4224 - i * 512)
                tps = pp0.tile([8, 512], F32, tag="tps")
                for k in range(10):
                    wt = p0.tile([P, 512], BF16, tag="wctt")
                    nc.sync.dma_start(wt[:, :nl], wct[k, :, n0:n0 + nl])
                    nc.tensor.matmul(tps[:nw, :nl], caug[:, k, :], wt[:, :nl],
                                     start=(k == 0), stop=(k == 9))
                nc.scalar.activation(tsb[:nw, n0:n0 + nl], tps[:nw, :nl],
                                     AF.Copy)
            nc.sync.dma_start(tdram[:, :], tsb[:nw, :])

        # ---- layernorm for a window pair -> PSUM broadcast [P, W2N] ------
        # acc-tile layout: [:, :N]+[:, N:] hold the two windows; returns one
        # [P, W2N] psum tile pair (rstd bcast, -mean*rstd bcast)
        def ln_pair(pool, rows, accp, fetch, tag):
            """fetch(s) -> [P, W2N] bf16 tile of the LN input, sub-tile s.
            Returns (rstd_bcast, -mean*rstd bcast) PSUM tiles."""
            ms0 = accp.tile([1, W2N], F32, tag="acc")
            ms1 = accp.tile([1, W2N], F32, tag="acc")
            for s in range(KC):
                xbs = fetch(s)
                xsq = pool.tile([P, W2N], BF16, tag=tag + "xsq")
                nc.vector.tensor_mul(xsq[:], xbs[:], xbs[:])
                nc.tensor.matmul(ms0[:], ones_c[:], xbs[:],
                                 start=(s == 0), stop=(s == KC - 1))
                nc.tensor.matmul(ms1[:], ones_c[:], xsq[:],
                                 start=(s == 0), stop=(s == KC - 1))
            mean = rows.tile([1, W2N], F32, tag="r_mean")
            ra = rows.tile([1, W2N], F32, tag="r_a")
            rb = rows.tile([1, W2N], F32, tag="r_b")
            nc.vector.tensor_scalar_mul(mean[:], ms0[:], 1.0 / C)
            nc.vector.tensor_scalar_mul(ra[:], ms1[:], 1.0 / C)   # E[x^2]
            nc.vector.tensor_mul(rb[:], mean[:], mean[:])         # mean^2
            nc.vector.tensor_sub(ra[:], ra[:], rb[:])             # var
            nc.scalar.activation(rb[:], ra[:], AF.Sqrt, bias=EPS) # sd
            nc.vector.reciprocal_approx_fast(ra[:], rb[:])        # 1/sd
            rstd = rows.tile([1, W2N], BF16, tag="r_rstd")
            nc.gpsimd.tensor_copy(rstd[:], ra[:])
            bneg = rows.tile([1, W2N], BF16, tag="r_bneg")
            nc.vector.scalar_tensor_tensor(
                bneg[:], mean[:], -1.0, rstd[:],
                mybir.AluOpType.mult, mybir.AluOpType.mult)
            bc = accp.tile([P, W2N], F32, tag="acc")
            nc.tensor.matmul(bc[:], ones_b[:1, :P], rstd[:],
                             start=True, stop=True)
            bb = accp.tile([P, W2N], F32, tag="acc")
            nc.tensor.matmul(bb[:], ones_b[:1, :P], bneg[:],
                             start=True, stop=True)
            return bc, bb

        # ==== attention superphase: per pair LN1 -> QKV -> attn -> proj ===
        with tc.tile_pool(name="sp", bufs=2) as sp, \
             tc.tile_pool(name="sp1", bufs=1) as sp1, \
             tc.tile_pool(name="spw", bufs=2) as spw, \
             tc.tile_pool(name="sps", bufs=3) as sps, \
             tc.tile_pool(name="spr", bufs=2) as spr, \
             tc.tile_pool(name="rows", bufs=1) as rows, \
             tc.tile_pool(name="accp", bufs=8, space="PSUM") as accp:
            def fetch_dram_bf16(src, w0, pool, tag):
                def fetch(s):
                    t = pool.tile([P, W2N], BF16, tag=tag)
                    for wh in range(2):
                        nc.sync.dma_start(t[:, wh * N:(wh + 1) * N],
                                          src[w0 + wh, :, s, :])
                    return t
                return fetch

            for pr in range(NPAIR):
                w0 = 2 * pr
                f_x = fetch_dram_bf16(xTb, w0, spw, "xbs")
                bc, bb = ln_pair(spw, rows, accp, f_x, "ln1")
                hw = sp.tile([P, KC, W2N], BF16, tag="hw")
                for s in range(KC):
                    xbs = f_x(s)
                    nc.vector.tensor_mul(hw[:, s, :], xbs[:], bc[:])
                    nc.vector.tensor_add(hw[:, s, :], hw[:, s, :], bb[:])
                # QK (96-stride packed), N = both windows
                qkst = sp1.tile([P, QKM, W2N], BF16, tag="qkst")
                for m in range(QKM):
                    wt = spw.tile([P, KC, P], BF16, tag="wqkt")
                    nc.sync.dma_start(wt[:], wqk[m])
                    t1m = spw.tile([1, 2, P], BF16, tag="t1m")
                    nc.sync.dma_start(
                        t1m[:], tdram[w0:w0 + 2, P * m:P * (m + 1)]
                        .unsqueeze(0))
                    qs = accp.tile([P, W2N], F32, tag="acc")
                    for k in range(KC):
                        nc.tensor.matmul(qs[:], wt[:, k, :], hw[:, k, :],
                                         start=(k == 0), stop=False)
                    nc.tensor.matmul(qs[:, :N], t1m[:1, 0, :],
                                     ones_b[:1, :N], start=False, stop=False)
                    nc.tensor.matmul(qs[:, N:], t1m[:1, 1, :],
                                     ones_b[:1, :N], start=False, stop=True)
                    nc.scalar.activation(qkst[:, m, :], qs[:], AF.Copy)
                # V token-major into per-head slots (ones in col 0)
                vsl = sp1.tile([P, 2, 2, H, 73], BF16, tag="vsl")
                nc.vector.memset(vsl[:, :, :, :, 0:1], 1.0)
                for nch in range(4):
                    wvt = spw.tile([P, KC, 288], BF16, tag="wvt")
                    nc.sync.dma_start(wvt[:], wv[nch])
                    t1vc = spw.tile([1, 2, 288], BF16, tag="t1vc")
                    nc.sync.dma_start(
                        t1vc[:],
                        tdram[w0:w0 + 2, 3072 + 288 * nch:3072 + 288 * (nch + 1)]
                        .unsqueeze(0))
                    for tch in range(4):       # token chunks of the pair
                        wh, ms = divmod(tch, 2)
                        vs = accp.tile([P, W2N], F32, tag="acc")
                        tsl = slice(tch * P, (tch + 1) * P)
                        for k in range(KC):
                            nc.tensor.matmul(vs[:, :288], hw[:, k, tsl],
                                             wvt[:, k, :],
                                             start=(k == 0), stop=False)
                        nc.tensor.matmul(
                            vs[:, :288], ones_b[:1, :P], t1vc[:1, wh, :],
                            start=False, stop=True)
                        nc.scalar.activation(
                            vsl[:, wh, ms, 4 * nch:4 * nch + 4, 1:73],
                            vs[:, :288].rearrange("p (h d) -> p h d", d=72),
                            AF.Copy)
                # attention, head-outer so expb loads once per pair
                ost = sp1.tile([P, H, W2N], BF16, tag="ost")
                nc.gpsimd.memset(ost[64:, :, :], 0.0)
                for h in range(H):
                    ebt = sps.tile([P, 2, N], BF16, tag="ebt")
                    nc.sync.dma_start(ebt[:], expb[h])
                    pieces = _qk_pieces(h)
                    for wh in range(2):
                        nsl = slice(wh * N, (wh + 1) * N)
                        pt = sps.tile([P, 2, N], BF16, tag="pt")
                        po = accp.tile([P, W2N], F32, tag="acc")
                        for ms in range(2):
                            ssp = accp.tile([P, W2N], F32, tag="acc")
                            msl = slice(wh * N + ms * P, wh * N + (ms + 1) * P)
                            for i, (sub, base, ln) in enumerate(pieces):
                                nc.tensor.matmul(
                                    ssp[:, :N],
                                    qkst[base:base + ln, KOFF + sub, msl],
                                    qkst[base:base + ln, sub, nsl],
                                    start=(i == 0),
                                    stop=(i == len(pieces) - 1),
                                    tile_position=(base, 0))
                            nc.scalar.activation(pt[:, ms, :], ssp[:, :N],
                                                 AF.Exp, scale=SCALE)
                            nc.vector.tensor_mul(pt[:, ms, :], pt[:, ms, :],
                                                 ebt[:, ms, :])
                        for ms in range(2):
                            nc.tensor.matmul(po[:73, :N],
                                             vsl[:, wh, ms, h, :],
                                             pt[:, ms, :],
                                             start=(ms == 0), stop=(ms == 1))
                        linv = spr.tile([1, N], F32, tag="linv")
                        nc.vector.reciprocal_approx_fast(linv[:], po[0:1, :N])
                        pbs = spr.tile([P, N], F32, tag="pbs")
                        nc.gpsimd.partition_broadcast(pbs[:73, :], linv[:],
                                                      channels=73)
                        nc.scalar.activation(ost[:73, h, nsl], po[:73, :N],
                                             AF.Copy)
                        nc.vector.tensor_mul(ost[:73, h, nsl],
                                             ost[:73, h, nsl], pbs[:73, :])
                # proj + residual -> xpd (fp32) + xpdb (bf16 shadow)
                for pc in range(KC):
                    wpt = spw.tile([P, H, P], BF16, tag="wpt")
                    nc.sync.dma_start(wpt[:], wps[pc])
                    yps = accp.tile([P, W2N], F32, tag="acc")
                    for h in range(H):
                        nc.tensor.matmul(yps[:], wpt[:, h, :], ost[:, h, :],
                                         start=(h == 0), stop=False)
                    nc.tensor.matmul(yps[:], bias2[:1, P * pc:P * (pc + 1)],
                                     ones_b[:1, :W2N], start=False, stop=True)
                    xres = spw.tile([P, 2, N], F32, tag="xres")
                    for wh in range(2):
                        nc.sync.dma_start(xres[:, wh, :],
                                          xT[w0 + wh, :, pc, :])
                    nc.vector.tensor_add(
                        xres[:], xres[:],
                        yps[:].rearrange("p (u n) -> p u n", n=N))
                    xrb = spw.tile([P, 2, N], BF16, tag="xrb")
                    nc.scalar.activation(
                        xrb[:].rearrange("p u n -> p (u n)"),
                        xres[:].rearrange("p u n -> p (u n)"), AF.Copy)
                    for wh in range(2):
                        nc.sync.dma_start(xpd[w0 + wh, :, pc, :],
                                          xres[:, wh, :])
                        nc.sync.dma_start(xpdb[w0 + wh, :, pc, :],
                                          xrb[:, wh, :])
                # LN2 from the bf16 shadow
                f_xp = fetch_dram_bf16(xpdb, w0, spw, "xpbs")
                bc2, bb2 = ln_pair(spw, rows, accp, f_xp, "ln2")
                hp = sp.tile([P, KC, W2N], BF16, tag="hw")
                for s in range(KC):
                    xbs = f_xp(s)
                    nc.vector.tensor_mul(hp[:, s, :], xbs[:], bc2[:])
                    nc.vector.tensor_add(hp[:, s, :], hp[:, s, :], bb2[:])
                # fc1 -> gelu -> h2a
                h2a = sp1.tile([P, M1T, W2N], BF16, tag="h2a")
                for m1 in range(M1T):
                    w1t = spw.tile([P, KC, P], BF16, tag="w1t")
                    nc.sync.dma_start(w1t[:], w1c[m1])
                    ps1 = accp.tile([P, W2N], F32, tag="acc")
                    for k in range(KC):
                        nc.tensor.matmul(ps1[:], w1t[:, k, :], hp[:, k, :],
                                         start=(k == 0), stop=(k == KC - 1))
                    h2c = h2a[:, m1, :]
                    if not sim_gelu:
                        nc.scalar.activation(h2c, ps1[:], AF.Gelu_apprx_tanh,
                                             bias=f1bs[:, m1:m1 + 1])
                    else:
                        u = rows.tile([P, W2N], F32, tag="gelu_u")
                        nc.vector.tensor_add(
                            u[:], ps1[:],
                            f1bs[:, m1:m1 + 1].to_broadcast((P, W2N)))
                        t3 = rows.tile([P, W2N], F32, tag="gelu_t3")
                        nc.vector.tensor_mul(t3[:], u[:], u[:])
                        nc.vector.tensor_mul(t3[:], t3[:], u[:])
                        nc.vector.scalar_tensor_tensor(
                            t3[:], t3[:], 0.044715, u[:],
                            mybir.AluOpType.mult, mybir.AluOpType.add)
                        nc.scalar.activation(t3[:], t3[:], AF.Tanh,
                                             scale=0.7978845608028654)
                        nc.vector.scalar_tensor_tensor(
                            t3[:], t3[:], 1.0, u[:],
                            mybir.AluOpType.add, mybir.AluOpType.mult)
                        nc.vector.tensor_scalar_mul(h2c, t3[:], 0.5)
                # fc2 + residual + output
                for pm in range(KC):
                    w2t = spw.tile([P, M1T, P], BF16, tag="w2t")
                    nc.sync.dma_start(w2t[:], w2[pm])
                    ps2 = accp.tile([P, W2N], F32, tag="acc")
                    for m1 in range(M1T):
                        nc.tensor.matmul(ps2[:], w2t[:, m1, :], h2a[:, m1, :],
                                         start=(m1 == 0), stop=False)
                    nc.tensor.matmul(
                        ps2[:], bias2[:1, C + P * pm:C + P * (pm + 1)],
                        ones_b[:1, :W2N], start=False, stop=True)
                    xps = spw.tile([P, 2, N], F32, tag="xps")
                    for wh in range(2):
                        nc.sync.dma_start(xps[:, wh, :],
                                          xpd[w0 + wh, :, pm, :])
                    ot = spw.tile([P, 2, N], F32, tag="ot")
                    nc.vector.tensor_add(
                        ot[:], xps[:],
                        ps2[:].rearrange("p (u n) -> p u n", n=N))
                    for wh in range(2):
                        nc.sync.dma_start(outT[w0 + wh, :, pm, :],
                                          ot[:, wh, :])

    nc.compile()
    return nc


# ---------------------------------------------------------------------------
# host side
# ---------------------------------------------------------------------------

def _qk_colmap():
    m = np.full(2 * H * HS, -1, np.int64)
    for h in range(H):
        m[HS * h:HS * h + DH] = np.arange(72 * h, 72 * h + 72)
        m[H * HS + HS * h:H * HS + HS * h + DH] = \
            np.arange(C + 72 * h, C + 72 * h + 72)
    return m


def _prep_core_inputs(x_c, c_c, wdict):
    """x_c: [nw, N, C], c_c: [nw, C] -> per-core input map"""
    nw = x_c.shape[0]
    xT = np.ascontiguousarray(
        x_c.transpose(0, 2, 1).reshape(nw, KC, P, N).transpose(
            0, 2, 1, 3)).astype(np.float32)
    caug = np.zeros((nw, 1280), np.float32)
    caug[:, :C] = c_c
    caug[:, C] = 1.0
    cT = np.ascontiguousarray(caug.T.reshape(10, P, nw)).astype(NPBF16)
    return {"xT": xT, "xTb": xT.astype(NPBF16), "cT": cT, **wdict}


def _prep_weights(qkv_w, qkv_b, qkvt_w, qkvt_b, rpb_table, rel_idx,
                  proj_w, proj_b, fc1_w, fc1_b, fc2_w, fc2_b):
    qkmap = _qk_colmap()
    amap = np.concatenate([qkmap, np.arange(2 * C, 3 * C)])  # 4224 cols
    valid = amap >= 0

    wct = np.zeros((1280, 4224), np.float32)
    wct[:C, valid] = qkvt_w[amap[valid], :].T
    wct[C, valid] = (qkv_b + qkvt_b)[amap[valid]]
    wct = wct.reshape(10, P, 4224).astype(NPBF16)

    nqk = 2 * H * HS
    wqkT = np.zeros((C, nqk), np.float32)
    wqkT[:, valid[:nqk]] = qkv_w[qkmap[valid[:nqk]], :].T
    wqk = np.ascontiguousarray(
        wqkT.reshape(KC, P, QKM, P).transpose(2, 1, 0, 3)).astype(NPBF16)

    wv = np.ascontiguousarray(
        qkv_w[2 * C:, :].T.reshape(KC, P, 4, 288).transpose(
            2, 1, 0, 3)).astype(NPBF16)

    bias = rpb_table[rel_idx]                      # [N(n), N(m), H]
    expb = np.ascontiguousarray(
        np.exp(bias).transpose(2, 1, 0).reshape(H, 2, P, N).transpose(
            0, 2, 1, 3)).astype(NPBF16)

    wp_sl = np.zeros((P, H, C), np.float32)        # [slot-row d, head, p]
    for h in range(H):
        wp_sl[1:73, h, :] = proj_w[:, 72 * h:72 * h + 72].T
    wps = np.ascontiguousarray(
        wp_sl.reshape(P, H, KC, P).transpose(2, 0, 1, 3)).astype(NPBF16)

    w1c = np.ascontiguousarray(
        fc1_w.T.reshape(KC, P, M1T, P).transpose(2, 1, 0, 3)).astype(NPBF16)
    w2 = np.ascontiguousarray(
        fc2_w.T.reshape(M1T, P, KC, P).transpose(2, 1, 0, 3)).astype(NPBF16)
    f1b = np.ascontiguousarray(fc1_b.reshape(M1T, P).T).astype(np.float32)
    b2 = np.concatenate([proj_b, fc2_b]).reshape(1, 2 * C).astype(NPBF16)

    return {"wct": wct, "wqk": wqk, "wv": wv, "expb": expb, "wps": wps,
            "w1c": w1c, "w2": w2, "f1b": f1b, "b2": b2}


_PROGRAM = None


def kernel(x, c, qkv_w, qkv_b, qkvt_w, qkvt_b, rpb_table, proj_w, proj_b,
           fc1_w, fc1_b, fc2_w, fc2_b, rel_idx, _trace=False):
    global _PROGRAM
    x = np.asarray(x, np.float32)
    c = np.asarray(c, np.float32)
    wdict = _prep_weights(
        np.asarray(qkv_w, np.float32), np.asarray(qkv_b, np.float32),
        np.asarray(qkvt_w, np.float32), np.asarray(qkvt_b, np.float32),
        np.asarray(rpb_table, np.float32), np.asarray(rel_idx),
        np.asarray(proj_w, np.float32), np.asarray(proj_b, np.float32),
        np.asarray(fc1_w, np.float32), np.asarray(fc1_b, np.float32),
        np.asarray(fc2_w, np.float32), np.asarray(fc2_b, np.float32))

    if _PROGRAM is None:
        _PROGRAM = build_program(NW)
    nc = _PROGRAM

    in_maps = []
    for core in range(NCORES):
        sl = slice(core * NW, (core + 1) * NW)
        in_maps.append(_prep_core_inputs(x[sl], c[sl], wdict))

    res = bass_utils.run_bass_kernel_spmd(
        nc, in_maps, core_ids=list(range(NCORES)), trace=_trace)

    out = np.empty((B, N, C), np.float32)
    for core in range(NCORES):
        oT = res.results[core]["outT"]            # [NW, P, KC, N]
        out[core * NW:(core + 1) * NW] = \
            oT.transpose(0, 2, 1, 3).reshape(NW, C, N).transpose(0, 2, 1)
    if _trace:
        return out, res
    return out



# revision 8
# speedup vs baseline: 1.3300x; 1.3300x over previous
"""DiffiT transformer block kernel for 8 Trainium2 NeuronCores.

Data-parallel over the B=64 window axis (8 windows per core, processed as
4 window-pairs so matmul free dims are 512). Activations are feature-major
([channel, token]); every linear contracts over the SBUF partition axis.
Q/K stay feature-major with heads packed at a 96-row stride so per-head
score matmuls can slice 32-aligned partition pieces; V is token-major into
per-head slots with a ones-column so the PV matmul also yields the softmax
denominator.

v2 restructure vs the baseline:
- All per-pair activations (x, hw, qkst, vsl, ost, xpd, h2a) live in SBUF;
  no DRAM round trips for xpd / bf16 shadows (x is loaded bf16 directly).
- Bias/conditioning fixups moved off the PE: QKV conditioning enters as a
  per-partition ACT bias column (T transposed on-chip once via DVE block
  transposes); proj/fc2 bias + residual are fused into one DVE
  scalar_tensor_tensor evacuation per chunk.
- Attention post-processing merged into large ops: one [128,512] exp per
  (head, window), one ebt multiply, one PV psum for both windows, softmax
  1/l normalization via GpSimd partition_broadcast + one DVE multiply.
- LN per-token scale/shift broadcasts moved from PE ones-matmuls to GpSimd
  partition_broadcast.
- Weight tiles stream through one shared 3-deep buffer ring (prefetch).
"""

import math
from contextlib import ExitStack

import numpy as np
import ml_dtypes

import concourse.bass as bass
import concourse.mybir as mybir
import concourse.tile as tile
from concourse import bacc
from concourse import bass_utils

F32 = mybir.dt.float32
F32R = mybir.dt.float32r
BF16 = mybir.dt.bfloat16
NPBF16 = ml_dtypes.bfloat16
AF = mybir.ActivationFunctionType
ALU = mybir.AluOpType

P = 128
WS = 16
N = 256            # tokens per window
C = 1152           # hidden
H = 16             # heads
DH = 72            # head dim
HS = 96            # head stride in the QK packing (32-aligned, >= DH)
MLP = 4608
EPS = 1e-6
B = 64
NCORES = 8
NW = B // NCORES   # windows per core
NPAIR = NW // 2
W2N = 2 * N        # tokens per window pair
KC = C // P        # 9  k-tiles over the hidden dim
QKM = 2 * H * HS // P   # 24 m-tiles over packed Q+K (96-stride)
KOFF = QKM // 2    # first K-side m-tile
M1T = MLP // P     # 36 fc1 row tiles
SCALE = 1.0 / math.sqrt(DH)


def _qk_pieces(h):
    """32-aligned partition pieces covering head h's 72 rows in the
    96-stride packing: [(subtile, base, length), ...]; piece legality:
    base 0 any len, base 64 len<=64, base 32/96 len<=32."""
    start, end = HS * h, HS * h + DH
    out = []
    while start < end:
        sub, base = divmod(start, P)
        ln = min(end - start, P - base)
        if base == 64:
            ln = min(ln, 64)
        elif base in (32, 96):
            ln = min(ln, 32)
        elif base != 0:
            raise AssertionError(base)
        out.append((sub, base, ln))
        start += ln
    return out


def build_program(npair=NPAIR):
    nc = bacc.Bacc("TRN2", target_bir_lowering=False, debug=False,
                   num_devices=NCORES)

    # register the layernorm epsilon as a const AP (activation float biases
    # other than 0.0/1.0 need one), same pattern as Bass.__init__
    eps_t = nc.alloc_sbuf_tensor("const-eps", [P, 1], F32)
    nc.gpsimd.memset(eps_t.ap(), EPS)
    nc.const_aps.aps[(F32, EPS)] = eps_t.ap()
    nc.all_engine_barrier()

    def din(name, shape, dt):
        return nc.dram_tensor(name, shape, dt, kind="ExternalInput").ap()

    xT = din("xT", [npair, P, KC, 2, N], BF16)   # x, feature-major, bf16
    cT = din("cT", [10, P, NW], BF16)            # c augmented with ones row
    wct = din("wct", [10, P, 4224], BF16)        # qkvt^T reordered + bias row
    wqk = din("wqk", [QKM, P, KC, P], BF16)      # qkv^T QK part, 96-stride
    wv = din("wv", [4, P, KC, 288], BF16)        # qkv^T V part, chunk-major
    expb = din("expb", [H, P, 2, N], BF16)       # exp(rel-pos bias)^T per head
    wps = din("wps", [KC, P, H, P], BF16)        # proj^T, head-slot padded
    w1c = din("w1c", [M1T, P, KC, P], BF16)      # fc1^T pre-chunked
    w2 = din("w2", [KC, P, M1T, P], BF16)        # fc2^T, pm-chunked
    f1b = din("f1b", [P, M1T], F32)              # fc1 bias, per-partition
    b2f = din("b2f", [P, KC, 2], F32)            # proj_b / fc2_b columns
    outT = nc.dram_tensor("outT", [npair, P, KC, 2, N], F32,
                          kind="ExternalOutput").ap()

    with tile.TileContext(nc) as tc, ExitStack() as ctx:
        keep = ctx.enter_context(tc.tile_pool(name="keep", bufs=1))
        dram = ctx.enter_context(tc.tile_pool(name="dram", bufs=1,
                                              space="DRAM"))

        ones_b = keep.tile([1, W2N], BF16, tag="ones_b")  # bf16 rhs of K=1
        ones_c = keep.tile([P, 1], BF16, tag="ones_c")    # lhsT of column sums
        nc.gpsimd.memset(ones_b[:], 1.0)
        nc.gpsimd.memset(ones_c[:], 1.0)
        b2s = keep.tile([P, KC, 2], F32, tag="b2s")
        nc.sync.dma_start(b2s[:], b2f[:])
        f1bs = keep.tile([P, M1T], F32, tag="f1bs")
        nc.sync.dma_start(f1bs[:], f1b[:])
        # T^T bias columns for the QK conditioning fixup: [feature, window]
        tq = keep.tile([P, QKM, 32], BF16, tag="tq")

        tdram = dram.tile([NW, 4224], BF16)

        # ---- phase 0: conditioning T = c_aug @ W_ct ----------------------
        with tc.tile_pool(name="ph0", bufs=2) as p0, \
             tc.tile_pool(name="ph0p", bufs=2, space="PSUM") as pp0:
            caug = p0.tile([P, 10, NW], BF16, tag="caug")
            nc.sync.dma_start(caug[:], cT.rearrange("k p w -> p k w"))
            tsb = p0.tile([32, 4224], BF16, tag="tsb")
            for i in range(9):
                n0, nl = i * 512, min(512, 4224 - i * 512)
                tps = pp0.tile([8, 512], F32, tag="tps")
                for k in range(10):
                    wt = p0.tile([P, 512], BF16, tag="wctt")
                    nc.sync.dma_start(wt[:, :nl], wct[k, :, n0:n0 + nl])
                    nc.tensor.matmul(tps[:NW, :nl], caug[:, k, :], wt[:, :nl],
                                     start=(k == 0), stop=(k == 9))
                nc.scalar.activation(tsb[:NW, n0:n0 + nl], tps[:NW, :nl],
                                     AF.Copy)
            nc.sync.dma_start(tdram[:, :], tsb[:NW, :])
            # transpose the QK part of T to feature-major bias columns:
            # tq[32g+p, m, j] = tsb[j, 128m + 32g + p]
            tsbm = tsb.rearrange("j (m c) -> j m c", c=P)
            for g in range(4):
                nc.vector.transpose(tq[32 * g:32 * (g + 1), :, :],
                                    tsbm[:32, :QKM, 32 * g:32 * (g + 1)])

        # ==== per window pair =============================================
        with tc.tile_pool(name="px", bufs=2) as px, \
             tc.tile_pool(name="ph", bufs=2) as ph, \
             tc.tile_pool(name="pbig", bufs=1) as pbig, \
             tc.tile_pool(name="pw", bufs=3) as pw, \
             tc.tile_pool(name="psm", bufs=3) as psm, \
             tc.tile_pool(name="psm2", bufs=2) as psm2, \
             tc.tile_pool(name="rows", bufs=1) as rows, \
             tc.tile_pool(name="pot", bufs=1) as pot, \
             tc.tile_pool(name="accp", bufs=8, space="PSUM") as accp:

            def ln_pair(chunk, dst, tag):
                """chunk(s): [P, W2N] bf16 AP of LN input, k-tile s.
                dst = LN(input) bf16. Stats via PE column-sum matmuls;
                per-token scale/shift broadcast on GpSimd."""
                ms0 = accp.tile([1, W2N], F32, tag="acc")
                ms1 = accp.tile([1, W2N], F32, tag="acc")
                for s in range(KC):
                    xbs = chunk(s)
                    xsq = psm.tile([P, W2N], BF16, tag="xsq")
                    nc.vector.tensor_mul(xsq[:], xbs, xbs)
                    nc.tensor.matmul(ms0[:], ones_c[:], xbs,
                                     start=(s == 0), stop=(s == KC - 1))
                    nc.tensor.matmul(ms1[:], ones_c[:], xsq[:],
                                     start=(s == 0), stop=(s == KC - 1))
                mean = rows.tile([1, W2N], F32, tag="r_mean")
                ra = rows.tile([1, W2N], F32, tag="r_a")
                rb = rows.tile([1, W2N], F32, tag="r_b")
                nc.vector.tensor_scalar_mul(mean[:], ms0[:], 1.0 / C)
                nc.vector.tensor_scalar_mul(ra[:], ms1[:], 1.0 / C)  # E[x^2]
                nc.vector.tensor_mul(rb[:], mean[:], mean[:])        # mean^2
                nc.vector.tensor_sub(ra[:], ra[:], rb[:])            # var
                nc.scalar.activation(rb[:], ra[:], AF.Sqrt, bias=EPS)
                nc.vector.reciprocal_approx_fast(ra[:], rb[:])       # 1/sd
                rstd = rows.tile([1, W2N], BF16, tag="r_rstd")
                nc.gpsimd.tensor_copy(rstd[:], ra[:])
                bneg = rows.tile([1, W2N], BF16, tag="r_bneg")
                nc.vector.scalar_tensor_tensor(
                    bneg[:], mean[:], -1.0, rstd[:], ALU.mult, ALU.mult)
                bcs = psm.tile([P, W2N], BF16, tag="bcs")
                bbs = psm.tile([P, W2N], BF16, tag="bbs")
                nc.gpsimd.partition_broadcast(bcs[:], rstd[:], channels=P)
                nc.gpsimd.partition_broadcast(bbs[:], bneg[:], channels=P)
                for s in range(KC):
                    xbs = chunk(s)
                    hc = dst[:, s, :]
                    nc.vector.tensor_mul(hc, xbs, bcs[:])
                    nc.vector.tensor_add(hc, hc, bbs[:])

            for pr in range(npair):
                w0 = 2 * pr
                xt = px.tile([P, KC, 2, N], BF16, tag="xt")
                nc.sync.dma_start(
                    xt[:].rearrange("p k u n -> p (k u n)"),
                    xT[pr].rearrange("p k u n -> p (k u n)"))
                hw = ph.tile([P, KC, W2N], BF16, tag="hw")
                ln_pair(lambda s: xt[:, s, :, :].rearrange("p u n -> p (u n)"),
                        hw, "ln1")

                # QK (96-stride packed), conditioning enters as ACT bias
                qkst = pbig.tile([P, QKM, W2N], BF16, tag="qkst")
                for m in range(QKM):
                    wt = pw.tile([P, M1T, P], BF16, tag="w")
                    nc.sync.dma_start(wt[:, :KC, :], wqk[m])
                    qs = accp.tile([P, W2N], F32, tag="acc")
                    for k in range(KC):
                        nc.tensor.matmul(qs[:], wt[:, k, :], hw[:, k, :],
                                         start=(k == 0), stop=(k == KC - 1))
                    for wh in range(2):
                        nc.scalar.activation(
                            qkst[:, m, wh * N:(wh + 1) * N],
                            qs[:, wh * N:(wh + 1) * N], AF.Identity,
                            bias=tq[:, m, w0 + wh:w0 + wh + 1])
                # V token-major into per-head slots (ones in col 0)
                vsl = pbig.tile([P, 2, 2, H, 73], BF16, tag="vsl")
                nc.vector.memset(vsl[:, :, :, :, 0:1], 1.0)
                for nch in range(4):
                    wvt = pw.tile([P, M1T, P], BF16, tag="w")
                    wvta = wvt[:].rearrange("p a b -> p (a b)")[:, :KC * 288] \
                        .rearrange("p (k n) -> p k n", n=288)
                    nc.sync.dma_start(wvta, wv[nch])
                    t1vc = psm.tile([1, 2, 288], BF16, tag="t1vc")
                    nc.sync.dma_start(
                        t1vc[:],
                        tdram[w0:w0 + 2, 3072 + 288 * nch:3072 + 288 * (nch + 1)]
                        .unsqueeze(0))
                    for tch in range(4):       # token chunks of the pair
                        wh, ms = divmod(tch, 2)
                        vs = accp.tile([P, W2N], F32, tag="acc")
                        tsl = slice(tch * P, (tch + 1) * P)
                        for k in range(KC):
                            nc.tensor.matmul(vs[:, :288], hw[:, k, tsl],
                                             wvta[:, k, :],
                                             start=(k == 0), stop=False)
                        nc.tensor.matmul(
                            vs[:, :288], ones_b[:1, :P], t1vc[:1, wh, :],
                            start=False, stop=True)
                        nc.scalar.activation(
                            vsl[:, wh, ms, 4 * nch:4 * nch + 4, 1:73],
                            vs[:, :288].rearrange("p (h d) -> p h d", d=72),
                            AF.Copy)
                # attention, head-outer, both windows per head
                ost = pbig.tile([P, H, W2N], BF16, tag="ost")
                nc.gpsimd.memset(ost[64:, :, :], 0.0)
                for h in range(H):
                    ebt = psm2.tile([P, 2, N], BF16, tag="ebt")
                    nc.sync.dma_start(ebt[:], expb[h])
                    pieces = _qk_pieces(h)
                    po = accp.tile([P, W2N], F32, tag="acc")
                    pts = []
                    for wh in range(2):
                        nsl = slice(wh * N, (wh + 1) * N)
                        pt = psm.tile([P, 2, N], BF16, tag="pt")
                        ssp = accp.tile([P, W2N], F32, tag="acc")
                        for ms in range(2):
                            msl = slice(wh * N + ms * P, wh * N + (ms + 1) * P)
                            osl = slice(ms * N, (ms + 1) * N)
                            for i, (sub, base, ln) in enumerate(pieces):
                                nc.tensor.matmul(
                                    ssp[:, osl],
                                    qkst[base:base + ln, KOFF + sub, msl],
                                    qkst[base:base + ln, sub, nsl],
                                    start=(i == 0),
                                    stop=(i == len(pieces) - 1),
                                    tile_position=(base, 0))
                        ptf = pt[:].rearrange("p u n -> p (u n)")
                        nc.scalar.activation(ptf, ssp[:], AF.Exp, scale=SCALE)
                        nc.vector.tensor_mul(
                            ptf, ptf, ebt[:].rearrange("p u n -> p (u n)"))
                        pts.append(pt)
                    for wh in range(2):
                        for ms in range(2):
                            nc.tensor.matmul(po[:73, wh * N:(wh + 1) * N],
                                             vsl[:, wh, ms, h, :],
                                             pts[wh][:, ms, :],
                                             start=(ms == 0), stop=(ms == 1))
                    linv = psm.tile([1, W2N], F32, tag="linv")
                    nc.vector.reciprocal_approx_fast(linv[:], po[0:1, :])
                    pbs = psm2.tile([P, W2N], F32, tag="pbs")
                    nc.gpsimd.partition_broadcast(pbs[:73, :], linv[:],
                                                  channels=73)
                    nc.vector.tensor_mul(ost[:73, h, :], po[:73, :],
                                         pbs[:73, :])
                # proj + bias + residual -> xpd (bf16, stays in SBUF)
                xpd = px.tile([P, KC, W2N], BF16, tag="xpd")
                for pc in range(KC):
                    wpt = pw.tile([P, M1T, P], BF16, tag="w")
                    nc.sync.dma_start(wpt[:, :H, :], wps[pc])
                    yps = accp.tile([P, W2N], F32, tag="acc")
                    for hh in range(H):
                        nc.tensor.matmul(yps[:], wpt[:, hh, :], ost[:, hh, :],
                                         start=(hh == 0), stop=(hh == H - 1))
                    nc.vector.scalar_tensor_tensor(
                        xpd[:, pc, :], yps[:], b2s[:, pc, 0:1],
                        xt[:, pc, :, :].rearrange("p u n -> p (u n)"),
                        ALU.add, ALU.add)
                # LN2 + fc1 + gelu
                hp = ph.tile([P, KC, W2N], BF16, tag="hw")
                ln_pair(lambda s: xpd[:, s, :], hp, "ln2")
                h2a = pbig.tile([P, M1T, W2N], BF16, tag="h2a")
                for m1 in range(M1T):
                    w1t = pw.tile([P, M1T, P], BF16, tag="w")
                    nc.sync.dma_start(w1t[:, :KC, :], w1c[m1])
                    ps1 = accp.tile([P, W2N], F32, tag="acc")
                    for k in range(KC):
                        nc.tensor.matmul(ps1[:], w1t[:, k, :], hp[:, k, :],
                                         start=(k == 0), stop=(k == KC - 1))
                    nc.scalar.activation(h2a[:, m1, :], ps1[:],
                                         AF.Gelu_apprx_tanh,
                                         bias=f1bs[:, m1:m1 + 1])
                # fc2 + bias + residual -> out
                for pm in range(KC):
                    w2t = pw.tile([P, M1T, P], BF16, tag="w")
                    nc.sync.dma_start(w2t[:], w2[pm])
                    ps2 = accp.tile([P, W2N], F32, tag="acc")
                    for m1 in range(M1T):
                        nc.tensor.matmul(ps2[:], w2t[:, m1, :], h2a[:, m1, :],
                                         start=(m1 == 0), stop=(m1 == M1T - 1))
                    ot = pot.tile([P, W2N], F32, tag="ot")
                    nc.vector.scalar_tensor_tensor(
                        ot[:], ps2[:], b2s[:, pm, 1:2], xpd[:, pm, :],
                        ALU.add, ALU.add)
                    nc.sync.dma_start(
                        outT[pr, :, pm].rearrange("p u n -> p (u n)"), ot[:])

    nc.compile()
    return nc


# ---------------------------------------------------------------------------
# host side
# ---------------------------------------------------------------------------

def _qk_colmap():
    m = np.full(2 * H * HS, -1, np.int64)
    for h in range(H):
        m[HS * h:HS * h + DH] = np.arange(72 * h, 72 * h + 72)
        m[H * HS + HS * h:H * HS + HS * h + DH] = \
            np.arange(C + 72 * h, C + 72 * h + 72)
    return m


def _prep_core_inputs(x_c, c_c, wdict):
    """x_c: [nw, N, C], c_c: [nw, C] -> per-core input map"""
    nw = x_c.shape[0]
    xT = np.ascontiguousarray(
        x_c.reshape(nw // 2, 2, N, C).transpose(0, 3, 1, 2)  # [pr, C, 2, N]
        .reshape(nw // 2, KC, P, 2, N).transpose(0, 2, 1, 3, 4)
    ).astype(NPBF16)
    caug = np.zeros((nw, 1280), np.float32)
    caug[:, :C] = c_c
    caug[:, C] = 1.0
    cT = np.ascontiguousarray(caug.T.reshape(10, P, nw)).astype(NPBF16)
    return {"xT": xT, "cT": cT, **wdict}


def _prep_weights(qkv_w, qkv_b, qkvt_w, qkvt_b, rpb_table, rel_idx,
                  proj_w, proj_b, fc1_w, fc1_b, fc2_w, fc2_b):
    qkmap = _qk_colmap()
    amap = np.concatenate([qkmap, np.arange(2 * C, 3 * C)])  # 4224 cols
    valid = amap >= 0

    wct = np.zeros((1280, 4224), np.float32)
    wct[:C, valid] = qkvt_w[amap[valid], :].T
    wct[C, valid] = (qkv_b + qkvt_b)[amap[valid]]
    wct = wct.reshape(10, P, 4224).astype(NPBF16)

    nqk = 2 * H * HS
    wqkT = np.zeros((C, nqk), np.float32)
    wqkT[:, valid[:nqk]] = qkv_w[qkmap[valid[:nqk]], :].T
    wqk = np.ascontiguousarray(
        wqkT.reshape(KC, P, QKM, P).transpose(2, 1, 0, 3)).astype(NPBF16)

    wv = np.ascontiguousarray(
        qkv_w[2 * C:, :].T.reshape(KC, P, 4, 288).transpose(
            2, 1, 0, 3)).astype(NPBF16)

    bias = rpb_table[rel_idx]                      # [N(n), N(m), H]
    expb = np.ascontiguousarray(
        np.exp(bias).transpose(2, 1, 0).reshape(H, 2, P, N).transpose(
            0, 2, 1, 3)).astype(NPBF16)

    wp_sl = np.zeros((P, H, C), np.float32)        # [slot-row d, head, p]
    for h in range(H):
        wp_sl[1:73, h, :] = proj_w[:, 72 * h:72 * h + 72].T
    wps = np.ascontiguousarray(
        wp_sl.reshape(P, H, KC, P).transpose(2, 0, 1, 3)).astype(NPBF16)

    w1c = np.ascontiguousarray(
        fc1_w.T.reshape(KC, P, M1T, P).transpose(2, 1, 0, 3)).astype(NPBF16)
    w2 = np.ascontiguousarray(
        fc2_w.T.reshape(M1T, P, KC, P).transpose(2, 1, 0, 3)).astype(NPBF16)
    f1b = np.ascontiguousarray(fc1_b.reshape(M1T, P).T).astype(np.float32)
    b2f = np.ascontiguousarray(
        np.stack([proj_b.reshape(KC, P).T, fc2_b.reshape(KC, P).T], axis=2)
    ).astype(np.float32)

    return {"wct": wct, "wqk": wqk, "wv": wv, "expb": expb, "wps": wps,
            "w1c": w1c, "w2": w2, "f1b": f1b, "b2f": b2f}


_PROGRAM = None


def kernel(x, c, qkv_w, qkv_b, qkvt_w, qkvt_b, rpb_table, proj_w, proj_b,
           fc1_w, fc1_b, fc2_w, fc2_b, rel_idx, _trace=False):
    global _PROGRAM
    x = np.asarray(x, np.float32)
    c = np.asarray(c, np.float32)
    wdict = _prep_weights(
        np.asarray(qkv_w, np.float32), np.asarray(qkv_b, np.float32),
        np.asarray(qkvt_w, np.float32), np.asarray(qkvt_b, np.float32),
        np.asarray(rpb_table, np.float32), np.asarray(rel_idx),
        np.asarray(proj_w, np.float32), np.asarray(proj_b, np.float32),
        np.asarray(fc1_w, np.float32), np.asarray(fc1_b, np.float32),
        np.asarray(fc2_w, np.float32), np.asarray(fc2_b, np.float32))

    if _PROGRAM is None:
        _PROGRAM = build_program(NPAIR)
    nc = _PROGRAM

    in_maps = []
    for core in range(NCORES):
        sl = slice(core * NW, (core + 1) * NW)
        in_maps.append(_prep_core_inputs(x[sl], c[sl], wdict))

    res = bass_utils.run_bass_kernel_spmd(
        nc, in_maps, core_ids=list(range(NCORES)), trace=_trace)

    out = np.empty((B, N, C), np.float32)
    for core in range(NCORES):
        oT = res.results[core]["outT"]            # [NPAIR, P, KC, 2, N]
        out[core * NW:(core + 1) * NW] = \
            oT.transpose(0, 3, 2, 1, 4).reshape(NW, C, N).transpose(0, 2, 1)
    if _trace:
        return out, res
    return out
